# revision 9
# baseline (speedup 1.0000x reference)
# kernel.py — MABSINK (Sinkhorn attention block) Trainium2 Bass kernel, v3.
# Self-contained: hardcodes shapes B=8, n=1024, dQ=dV=512, H=8; shards batch
# across 8 NeuronCores (1 batch element per core), runs SPMD, gathers output.
#
# Math (per core, per head h; Q_h = (Q @ Wq.T + bq)[:, h*64:(h+1)*64]):
#   S   = Q_h Q_h^T / sqrt(512)            (symmetric!)
#   E   = exp(S);  r_i = sum_j E_ij;  c_i = sum_n E_in * invr_n (by symmetry)
#   A   = n*mu' * E_ij * invr_i * invc_j,  mu' = 1/n + 1e-8
#   O_h = Q_h + A @ Q_h
# then head-recombine -> LN0 -> x + relu(x@Wo.T+bo) -> LN1.
#
# v3: all input/intermediate transposes via xbar DMA-transpose (bf16),
# freeing the PE; c via DVE tensor_tensor_reduce against invr_rep (no PE
# matvec, no layout flip); invr_rep via [8,128]->[1,1024] DMA reshape +
# K=1 ones matmuls; accumulation chains interleave PSUM banks; head loop
# software-pipelined (E of head h+1 emitted around the A@Q of head h).

import math
import numpy as np

B, N, DQ, DV, H = 8, 1024, 512, 512, 8
D = DV // H          # 64 head dim
P = 128
NRC = N // P         # 8 row chunks
NCC = DV // P        # 4 feature chunks
LN_EPS = 1e-5
SCALE_S = 1.0 / math.sqrt(DV)
AFACT = N * (1.0 / N + 1e-8)   # n * mu'

_CACHE = {}


def _build(reps=1):
    import concourse.mybir as mybir
    from concourse import bacc
    import concourse.tile as tile
    from concourse.masks import make_identity
    from contextlib import ExitStack

    f32 = mybir.dt.float32
    bf = mybir.dt.bfloat16
    AF = mybir.ActivationFunctionType
    OP = mybir.AluOpType
    AX = mybir.AxisListType

    nc = bacc.Bacc()
    dQ = nc.dram_tensor("Q", [N, DQ], f32, kind="ExternalInput")
    dWq = nc.dram_tensor("Wq", [DV, DQ], f32, kind="ExternalInput")
    dbq = nc.dram_tensor("bq", [DQ], f32, kind="ExternalInput")
    dWo = nc.dram_tensor("Wo", [DV, DV], f32, kind="ExternalInput")
    dbo = nc.dram_tensor("bo", [DV], f32, kind="ExternalInput")
    dg0 = nc.dram_tensor("g0", [DV], f32, kind="ExternalInput")
    db0 = nc.dram_tensor("b0", [DV], f32, kind="ExternalInput")
    dg1 = nc.dram_tensor("g1", [DV], f32, kind="ExternalInput")
    db1 = nc.dram_tensor("b1", [DV], f32, kind="ExternalInput")
    dout = nc.dram_tensor("out", [N, DV], f32, kind="ExternalOutput")

    with tile.TileContext(nc) as tc, ExitStack() as ctx:
        pc = ctx.enter_context(tc.tile_pool(name="pc", bufs=1))
        pq = ctx.enter_context(tc.tile_pool(name="pq", bufs=2))
        pqt = ctx.enter_context(tc.tile_pool(name="pqt", bufs=4))
        pw = ctx.enter_context(tc.tile_pool(name="pw", bufs=4))
        pqp = ctx.enter_context(tc.tile_pool(name="pqp", bufs=4))
        pqptb = ctx.enter_context(tc.tile_pool(name="pqptb", bufs=4))
        pE = ctx.enter_context(tc.tile_pool(name="pE", bufs=18))
        pot = ctx.enter_context(tc.tile_pool(name="pot", bufs=4))
        po1 = ctx.enter_context(tc.tile_pool(name="po1", bufs=4))
        po2 = ctx.enter_context(tc.tile_pool(name="po2", bufs=4))
        psm = ctx.enter_context(tc.tile_pool(name="psm", bufs=2))
        prep = ctx.enter_context(tc.tile_pool(name="prep", bufs=2))
        pout = ctx.enter_context(tc.tile_pool(name="pout", bufs=2))

        # PSUM: eps 2x[128,1024]f32 (4 banks) + big 1x[128,1024]f32 (2)
        #       + sm 2x<=[128,512]f32 (2)  = 8 banks
        ps_e = ctx.enter_context(tc.tile_pool(name="ps_e", bufs=2, space="PSUM"))
        ps_big = ctx.enter_context(tc.tile_pool(name="ps_big", bufs=1, space="PSUM"))
        ps_sm = ctx.enter_context(tc.tile_pool(name="ps_sm", bufs=2, space="PSUM"))

        # ---- constants -------------------------------------------------
        ident = pc.tile([P, P], f32, tag="ident")
        make_identity(nc, ident)
        ident_b = pc.tile([P, P], bf, tag="ident_b")
        nc.vector.tensor_copy(ident_b, ident)
        ones_col = pc.tile([P, 1], bf, tag="ones_col")
        nc.vector.memset(ones_col, 1.0)
        ones_one = pc.tile([1, P], bf, tag="ones_one")
        nc.vector.memset(ones_one, 1.0)
        zero_col = pc.tile([P, 1], f32, tag="zero_col")
        nc.vector.memset(zero_col, 0.0)
        eps_col = pc.tile([P, 1], f32, tag="eps_col")
        nc.vector.memset(eps_col, LN_EPS)
        nc.const_aps.aps[(f32, 0.0)] = zero_col
        nc.const_aps.aps[(f32, LN_EPS)] = eps_col
        # SEL[p, c*128+m] = (p == c): replicates row c of an [8,128] rhs
        # across all 128 output partitions via matmul.
        sel = pc.tile([NRC, NRC * P], bf, tag="sel")
        nc.gpsimd.memset(sel, 0.0)
        nc.gpsimd.affine_select(
            out=sel.rearrange("p (c m) -> p c m", m=P),
            in_=sel.rearrange("p (c m) -> p c m", m=P),
            compare_op=mybir.AluOpType.not_equal,
            fill=1.0, base=0,
            pattern=[[-1, NRC], [0, P]],
            channel_multiplier=1,
        )

        def col_vec(dvec, tag):
            v4 = pc.tile([NCC, P], f32, tag=tag + "4")
            nc.sync.dma_start(v4, dvec.rearrange("(c p) -> c p", p=P))
            pst = ps_sm.tile([P, DV], f32, tag="sm", name="pst")
            nc.tensor.transpose(pst[:, :NCC], v4, ident[:NCC, :NCC])
            col = pc.tile([P, NCC], f32, tag=tag + "c")
            nc.vector.tensor_copy(col, pst[:, :NCC])
            return col

        g0_col = col_vec(dg0, "g0")
        b0_col = col_vec(db0, "b0")
        bo_col = col_vec(dbo, "bo")

        def row_vec(dvec, tag):
            rowf = pc.tile([1, DV], f32, tag=tag + "rf")
            nc.sync.dma_start(rowf, dvec[None])
            row = pc.tile([1, DV], bf, tag=tag + "r")
            nc.vector.tensor_copy(row, rowf)
            return row

        # replicated rows [128, DV] via K=1 ones matmul
        def repl_vec(dvec, tag, dt):
            row = row_vec(dvec, tag)
            ps = ps_big.tile([P, N], f32, tag="big", name="repl_ps")
            nc.tensor.matmul(ps[:, :DV], ones_one, row, start=True, stop=True)
            rep = pc.tile([P, DV], dt, tag=tag + "rep")
            nc.vector.tensor_copy(rep, ps[:, :DV])
            return rep

        g1_rep = repl_vec(dg1, "g1v", bf)
        b1_rep = repl_vec(db1, "b1v", f32)
        # bq replicated twice along free: [128, 1024] bf
        bq_row = row_vec(dbq, "bqv")
        bq_ps = ps_big.tile([P, N], f32, tag="big", name="bq_ps")
        nc.tensor.matmul(bq_ps[:, :DV], ones_one, bq_row, start=True, stop=True)
        nc.tensor.matmul(bq_ps[:, DV:], ones_one, bq_row, start=True, stop=True)
        bq_rep2 = pc.tile([P, N], bf, tag="bq_rep2")
        nc.vector.tensor_copy(bq_rep2, bq_ps)

        for _rep in range(reps):
            # ---- load Wq + Q, cast bf16, transpose via xbar DMA ------------
            WqT = [pw.tile([P, DV], bf, tag="wqt", name="wqt") for _ in range(NCC)]
            WoT = [pw.tile([P, DV], bf, tag="wot", name="wot") for _ in range(NCC)]
            QT = [pqt.tile([P, N], bf, tag="qt", name="qt") for _ in range(NCC)]

            for rc in range(NCC):
                wsb = pq.tile([P, DQ], f32, tag="qsb", name="wsb")
                nc.sync.dma_start(wsb, dWq[rc * P:(rc + 1) * P, :])
                wbf = pq.tile([P, DQ], bf, tag="wq_bf", bufs=4, name="wbf")
                nc.vector.tensor_copy(wbf, wsb)
                for kc in range(NCC):
                    nc.sync.dma_start_transpose(
                        WqT[kc][:, rc * P:(rc + 1) * P],
                        wbf[:, kc * P:(kc + 1) * P])

            for rc in range(NRC):
                qsb = pq.tile([P, DQ], f32, tag="qsb", name="qsb")
                nc.sync.dma_start(qsb, dQ[rc * P:(rc + 1) * P, :])
                qbf = pq.tile([P, DQ], bf, tag="q_bf", bufs=8, name="qbf")
                nc.vector.tensor_copy(qbf, qsb)
                for kc in range(NCC):
                    nc.sync.dma_start_transpose(
                        QT[kc][:, rc * P:(rc + 1) * P],
                        qbf[:, kc * P:(kc + 1) * P])

            # ---- Qp = Q@Wq.T + bq (bf, 4 row-pair tiles [128, 1024]) -------
            Qp = [pqp.tile([P, N], bf, tag="qp", name="qp") for _ in range(NCC)]
            for pr in range(NCC):
                ps = ps_e.tile([P, N], f32, tag="eps", name="qp_ps")
                for kc in range(NCC):
                    for half in range(2):
                        rc = 2 * pr + half
                        sl = slice(half * DV, (half + 1) * DV)
                        nc.tensor.matmul(ps[:, sl],
                                         QT[kc][:, rc * P:(rc + 1) * P], WqT[kc],
                                         start=(kc == 0), stop=(kc == NCC - 1))
                nc.vector.tensor_tensor(Qp[pr], ps, bq_rep2, OP.add)

            def qp_slice(jc, lo, hi):
                return Qp[jc // 2][:, (jc % 2) * DV + lo:(jc % 2) * DV + hi]

            # ---- QpT via xbar DMA-transpose --------------------------------
            QpTb = [pqptb.tile([P, N], bf, tag="qptb", name="qptb")
                    for _ in range(NCC)]
            for cc in range(NCC):
                for rc in range(NRC):
                    nc.sync.dma_start_transpose(
                        QpTb[cc][:, rc * P:(rc + 1) * P],
                        qp_slice(rc, cc * P, (cc + 1) * P))

            # ---- Wo loads (needed only by FFN) -----------------------------
            for rc in range(NCC):
                wsb = pq.tile([P, DQ], f32, tag="qsb", name="wsb2")
                nc.sync.dma_start(wsb, dWo[rc * P:(rc + 1) * P, :])
                wbf = pq.tile([P, DQ], bf, tag="wo_bf", bufs=4, name="wbf2")
                nc.vector.tensor_copy(wbf, wsb)
                for kc in range(NCC):
                    nc.sync.dma_start_transpose(
                        WoT[kc][:, rc * P:(rc + 1) * P],
                        wbf[:, kc * P:(kc + 1) * P])

            # ---- OT accumulator (transposed head outputs + residual) -------
            OT = [pot.tile([P, N], bf, tag="ot", name="ot") for _ in range(NCC)]

            # ---- per-head Sinkhorn attention (software-pipelined) ----------
            e_tiles = {}   # h -> (E chunks, r_mat)

            def emit_E(h, lo, hi):
                tb, po = h // 2, (h % 2) * D
                qht = QpTb[tb][po:po + D, :]
                if h not in e_tiles:
                    E = [pE.tile([P, N], bf, tag="E", name="E") for _ in range(NRC)]
                    r_mat = psm.tile([P, NRC], f32, tag="r_mat", name="r_mat")
                    e_tiles[h] = (E, r_mat)
                E, r_mat = e_tiles[h]
                for ci in range(lo, hi):
                    ps = ps_e.tile([P, N], f32, tag="eps", name="e_ps")
                    for hf in range(2):
                        nc.tensor.matmul(ps[:, hf * DV:(hf + 1) * DV],
                                         qht[:, ci * P:(ci + 1) * P],
                                         qht[:, hf * DV:(hf + 1) * DV],
                                         start=True, stop=True)
                    nc.scalar.activation(E[ci], ps, AF.Exp, scale=SCALE_S,
                                         accum_out=r_mat[:, ci:ci + 1])

            emit_E(0, 0, NRC)
            for h in range(H):
                tb, po = h // 2, (h % 2) * D
                E, r_mat = e_tiles.pop(h)

                if h + 1 < H:
                    emit_E(h + 1, 0, 4)

                # invr (partition) -> invr_rep [128, 1024] bf (free, replicated)
                invr = psm.tile([P, NRC], f32, tag="invr")
                nc.vector.reciprocal(invr, r_mat)
                invr_b = psm.tile([P, NRC], bf, tag="invr_b")
                nc.vector.tensor_copy(invr_b, invr)
                pfl = ps_sm.tile([NRC, P], bf, tag="sm", name="pfl")
                nc.tensor.transpose(pfl, invr_b, ident_b)
                sbt = psm.tile([NRC, P], bf, tag="sbt", bufs=8)
                nc.vector.tensor_copy(sbt, pfl)
                rep_ps = ps_big.tile([P, N], f32, tag="big", name="rep_ps")
                for c in range(NRC):
                    nc.tensor.matmul(rep_ps[:, c * P:(c + 1) * P],
                                     sel[:, c * P:(c + 1) * P], sbt,
                                     start=True, stop=True)
                invr_rep = prep.tile([P, N], bf, tag="irep", name="invr_rep")
                nc.scalar.activation(invr_rep[:, :DV], rep_ps[:, :DV], AF.Copy)
                nc.vector.tensor_copy(invr_rep[:, DV:], rep_ps[:, DV:])

                if h + 1 < H:
                    emit_E(h + 1, 4, NRC)

                # E[jc] *= invr_rep in place (so A@Q absorbs invr);
                # accum gives c_jc per chunk. Pipeline: recip + Qc + A@Q
                # accumulation step chase each chunk.
                c_mat = psm.tile([P, NRC], f32, tag="c_mat")
                invc = psm.tile([P, NRC], f32, tag="invc")
                Qc = [psm.tile([P, D], bf, tag=f"qc{jc}", name=f"qc{jc}")
                      for jc in range(NRC)]
                aq = ps_big.tile([P, N], f32, tag="big", name="aq_ps")
                for jc in range(NRC):
                    nc.vector.scalar_tensor_tensor(
                        E[jc], E[jc], 1.0, invr_rep, OP.mult, OP.mult,
                        accum_out=c_mat[:, jc:jc + 1])
                    nc.vector.reciprocal(invc[:, jc:jc + 1],
                                         c_mat[:, jc:jc + 1])
                    nc.vector.tensor_scalar(Qc[jc],
                                            qp_slice(jc, h * D, (h + 1) * D),
                                            invc[:, jc:jc + 1], AFACT,
                                            OP.mult, OP.mult)
                    for hf in range(2):
                        nc.tensor.matmul(aq[po:po + D, hf * DV:(hf + 1) * DV],
                                         Qc[jc], E[jc][:, hf * DV:(hf + 1) * DV],
                                         start=(jc == 0), stop=(jc == NRC - 1))
                nc.vector.tensor_tensor(OT[tb][po:po + D, :], aq[po:po + D, :],
                                        QpTb[tb][po:po + D, :], OP.add)

            # ---- LN0: per-token stats over features (M=1 ones-matvecs) -----
            sums = ps_e.tile([P, N], f32, tag="eps", name="ln0_sums")
            sqs = ps_e.tile([P, N], f32, tag="eps", name="ln0_sqs")
            for tb in range(NCC):
                sqh = pE.tile([P, N], bf, tag="E", name="sqh")
                nc.vector.tensor_tensor(sqh, OT[tb], OT[tb], OP.mult)
                for hf in range(2):
                    sl = slice(hf * DV, (hf + 1) * DV)
                    nc.tensor.matmul(sums[:1, sl], ones_col, OT[tb][:, sl],
                                     start=(tb == 0), stop=(tb == NCC - 1))
                    nc.tensor.matmul(sqs[:1, sl], ones_col, sqh[:, sl],
                                     start=(tb == 0), stop=(tb == NCC - 1))
            sum_row = psm.tile([1, N], f32, tag="sum_row")
            nc.vector.tensor_copy(sum_row, sums[:1, :])
            sq_row = psm.tile([1, N], f32, tag="sq_row")
            nc.scalar.activation(sq_row, sqs[:1, :], AF.Copy)
            s8 = psm.tile([NRC, P], f32, tag="s8")
            nc.sync.dma_start(s8, sum_row.rearrange("o (a b) -> o a b", a=NRC))
            q8 = psm.tile([NRC, P], f32, tag="q8")
            nc.sync.dma_start(q8, sq_row.rearrange("o (a b) -> o a b", a=NRC))
            pfs = ps_sm.tile([P, 2 * NRC], f32, tag="sm", name="pfs")
            nc.tensor.transpose(pfs[:, :NRC], s8, ident[:NRC, :NRC])
            nc.tensor.transpose(pfs[:, NRC:], q8, ident[:NRC, :NRC])
            m8 = psm.tile([P, NRC], f32, tag="m8")
            nc.vector.tensor_scalar_mul(m8, pfs[:, :NRC], 1.0 / DV)
            v8 = psm.tile([P, NRC], f32, tag="v8")
            nc.vector.tensor_scalar_mul(v8, pfs[:, NRC:], 1.0 / DV)
            t8 = psm.tile([P, NRC], f32, tag="t8")
            nc.vector.tensor_tensor(t8, m8, m8, OP.mult)
            nc.vector.tensor_tensor(v8, v8, t8, OP.subtract)
            sd8 = psm.tile([P, NRC], f32, tag="sd8")
            nc.scalar.activation(sd8, v8, AF.Sqrt, bias=LN_EPS)
            rstd8 = psm.tile([P, NRC], f32, tag="rstd8")
            nc.vector.reciprocal(rstd8, sd8)
            mr8 = psm.tile([P, NRC], bf, tag="mr8")
            nc.vector.tensor_tensor(mr8, m8, rstd8, OP.mult)
            rstd8_b = psm.tile([P, NRC], bf, tag="rstd8b")
            nc.vector.tensor_copy(rstd8_b, rstd8)
            # replicate rstd and m*rstd to [128, 1024] bf (flip + K=1 matmul)
            reps_sb = []
            for idx, src in enumerate((rstd8_b, mr8)):
                pfl2 = ps_sm.tile([NRC, P], bf, tag="sm", name="pfl2")
                nc.tensor.transpose(pfl2, src, ident_b)
                sbt2 = psm.tile([NRC, P], bf, tag=f"sbt{idx}")
                nc.vector.tensor_copy(sbt2, pfl2)
                row2 = psm.tile([1, N], bf, tag=f"row{idx}")
                nc.sync.dma_start(
                    row2.rearrange("o (a b) -> o a b", a=NRC), sbt2)
                rps = ps_big.tile([P, N], f32, tag="big", name="rps")
                nc.tensor.matmul(rps[:, :DV], ones_one, row2[:, :DV],
                                 start=True, stop=True)
                nc.tensor.matmul(rps[:, DV:], ones_one, row2[:, DV:],
                                 start=True, stop=True)
                rep = prep.tile([P, N], bf, tag="irep", name=f"rep{idx}")
                nc.scalar.activation(rep[:, :DV], rps[:, :DV], AF.Copy)
                nc.vector.tensor_copy(rep[:, DV:], rps[:, DV:])
                reps_sb.append(rep)
            rstd_rep, mr_rep = reps_sb

            # normalize: O1T = (OT*rstd - m*rstd)*g0 + b0   (g0/b0 on parts)
            O1T = [po1.tile([P, N], bf, tag="o1t", name="o1t")
                   for _ in range(NCC)]
            for cc in range(NCC):
                t = pE.tile([P, N], bf, tag="E", name="ln0t")
                nc.vector.tensor_tensor(t, OT[cc], rstd_rep, OP.mult)
                nc.vector.tensor_tensor(t, t, mr_rep, OP.subtract)
                nc.vector.tensor_scalar(O1T[cc], t, g0_col[:, cc:cc + 1],
                                        b0_col[:, cc:cc + 1], OP.mult, OP.add)

            # ---- FFN: O2T = O1T + relu(Wo@O1T + bo) ------------------------
            O2T = [po2.tile([P, N], bf, tag="o2t", name="o2t")
                   for _ in range(NCC)]
            for c2 in range(NCC):
                ps = ps_e.tile([P, N], f32, tag="eps", name="ffn_ps")
                for cc in range(NCC):
                    for hf in range(2):
                        nc.tensor.matmul(ps[:, hf * DV:(hf + 1) * DV],
                                         WoT[cc][:, c2 * P:(c2 + 1) * P],
                                         O1T[cc][:, hf * DV:(hf + 1) * DV],
                                         start=(cc == 0), stop=(cc == NCC - 1))
                t = pq.tile([P, N], bf, tag="ffn_t", name="ffn_t")
                nc.scalar.activation(t, ps, AF.Relu, bias=bo_col[:, c2:c2 + 1])
                nc.vector.tensor_tensor(O2T[c2], t, O1T[c2], OP.add)

            # ---- LN1 (row-major after xbar DMA-transpose) + store ----------
            for rc in range(NRC):
                psf = pq.tile([P, DV], bf, tag="psf", bufs=8, name="ln1_psf")
                for cc in range(NCC):
                    nc.sync.dma_start_transpose(
                        psf[:, cc * P:(cc + 1) * P],
                        O2T[cc][:, rc * P:(rc + 1) * P])
                mean = psm.tile([P, 1], f32, tag="ln1_mean")
                nc.vector.tensor_reduce(mean, psf, AX.X, OP.add)
                nc.vector.tensor_scalar_mul(mean, mean, 1.0 / DV)
                xc = pout.tile([P, DV], bf, tag="ln1_xc")
                nc.vector.tensor_scalar_sub(xc, psf, mean)
                sqj = pq.tile([P, DV], bf, tag="ffn_t", name="ln1_sqj")
                ss = psm.tile([P, 1], f32, tag="ln1_ss")
                nc.vector.scalar_tensor_tensor(sqj, xc, 1.0, xc,
                                               OP.mult, OP.mult, accum_out=ss)
                nc.vector.tensor_scalar_mul(ss, ss, 1.0 / DV)
                sd = psm.tile([P, 1], f32, tag="ln1_sd")
                nc.scalar.activation(sd, ss, AF.Sqrt, bias=LN_EPS)
                rstd = psm.tile([P, 1], f32, tag="ln1_rstd")
                nc.vector.reciprocal(rstd, sd)
                ob = pout.tile([P, DV], f32, tag="ln1_out")
                nc.vector.scalar_tensor_tensor(ob, xc, rstd, g1_rep,
                                               OP.mult, OP.mult)
                nc.gpsimd.tensor_tensor(ob, ob, b1_rep, OP.add)
                nc.sync.dma_start(dout[rc * P:(rc + 1) * P, :], ob)

    nc.finalize()
    return nc


def kernel(**inputs):
    from concourse.bass_utils import run_bass_kernel_spmd

    if "nc" not in _CACHE:
        _CACHE["nc"] = _build()
    nc = _CACHE["nc"]

    Q = np.ascontiguousarray(np.asarray(inputs["Q"], dtype=np.float32))
    shared = {k: np.ascontiguousarray(np.asarray(inputs[k], dtype=np.float32))
              for k in ("Wq", "bq", "Wo", "bo", "g0", "b0", "g1", "b1")}
    in_maps = [dict(Q=np.ascontiguousarray(Q[b]), **shared) for b in range(B)]

    res = run_bass_kernel_spmd(nc, in_maps, core_ids=list(range(B)),
                               **_CACHE.get("run_kwargs", {}))
    _CACHE["last_result"] = res
    return np.stack([r["out"] for r in res.results], axis=0)


# revision 10
# speedup vs baseline: 1.5979x; 1.5979x over previous
# kernel.py — MABSINK (Sinkhorn attention block) Trainium2 Bass kernel, v3.
# Self-contained: hardcodes shapes B=8, n=1024, dQ=dV=512, H=8; shards batch
# across 8 NeuronCores (1 batch element per core), runs SPMD, gathers output.
#
# Math (per core, per head h; Q_h = (Q @ Wq.T + bq)[:, h*64:(h+1)*64]):
#   S   = Q_h Q_h^T / sqrt(512)            (symmetric!)
#   E   = exp(S);  r_i = sum_j E_ij;  c_i = sum_n E_in * invr_n (by symmetry)
#   A   = n*mu' * E_ij * invr_i * invc_j,  mu' = 1/n + 1e-8
#   O_h = Q_h + A @ Q_h
# then head-recombine -> LN0 -> x + relu(x@Wo.T+bo) -> LN1.
#
# v3: all input/intermediate transposes via xbar DMA-transpose (bf16),
# freeing the PE; c via DVE tensor_tensor_reduce against invr_rep (no PE
# matvec, no layout flip); invr_rep via [8,128]->[1,1024] DMA reshape +
# K=1 ones matmuls; accumulation chains interleave PSUM banks; head loop
# software-pipelined (E of head h+1 emitted around the A@Q of head h).

import math
import numpy as np

B, N, DQ, DV, H = 8, 1024, 512, 512, 8
D = DV // H          # 64 head dim
P = 128
NRC = N // P         # 8 row chunks
NCC = DV // P        # 4 feature chunks
LN_EPS = 1e-5
SCALE_S = 1.0 / math.sqrt(DV)
AFACT = N * (1.0 / N + 1e-8)   # n * mu'

_CACHE = {}


def _build(reps=1):
    import concourse.mybir as mybir
    from concourse import bacc
    import concourse.tile as tile
    from concourse.masks import make_identity
    from contextlib import ExitStack

    f32 = mybir.dt.float32
    bf = mybir.dt.bfloat16
    AF = mybir.ActivationFunctionType
    OP = mybir.AluOpType
    AX = mybir.AxisListType

    nc = bacc.Bacc()
    dQ = nc.dram_tensor("Q", [N, DQ], f32, kind="ExternalInput")
    dWq = nc.dram_tensor("Wq", [DV, DQ], f32, kind="ExternalInput")
    dbq = nc.dram_tensor("bq", [DQ], f32, kind="ExternalInput")
    dWo = nc.dram_tensor("Wo", [DV, DV], f32, kind="ExternalInput")
    dbo = nc.dram_tensor("bo", [DV], f32, kind="ExternalInput")
    dg0 = nc.dram_tensor("g0", [DV], f32, kind="ExternalInput")
    db0 = nc.dram_tensor("b0", [DV], f32, kind="ExternalInput")
    dg1 = nc.dram_tensor("g1", [DV], f32, kind="ExternalInput")
    db1 = nc.dram_tensor("b1", [DV], f32, kind="ExternalInput")
    dout = nc.dram_tensor("out", [N, DV], f32, kind="ExternalOutput")

    with tile.TileContext(nc) as tc, ExitStack() as ctx:
        pc = ctx.enter_context(tc.tile_pool(name="pc", bufs=1))
        pq = ctx.enter_context(tc.tile_pool(name="pq", bufs=2))
        pqt = ctx.enter_context(tc.tile_pool(name="pqt", bufs=4))
        pw = ctx.enter_context(tc.tile_pool(name="pw", bufs=4))
        pqp = ctx.enter_context(tc.tile_pool(name="pqp", bufs=4))
        pqptb = ctx.enter_context(tc.tile_pool(name="pqptb", bufs=4))
        pE = ctx.enter_context(tc.tile_pool(name="pE", bufs=18))
        pot = ctx.enter_context(tc.tile_pool(name="pot", bufs=4))
        po1 = ctx.enter_context(tc.tile_pool(name="po1", bufs=4))
        po2 = ctx.enter_context(tc.tile_pool(name="po2", bufs=4))
        psm = ctx.enter_context(tc.tile_pool(name="psm", bufs=2))
        prep = ctx.enter_context(tc.tile_pool(name="prep", bufs=2))
        pout = ctx.enter_context(tc.tile_pool(name="pout", bufs=2))

        # PSUM: eps 2x[128,1024]f32 (4 banks) + big 1x[128,1024]f32 (2)
        #       + sm 2x<=[128,512]f32 (2)  = 8 banks
        ps_e = ctx.enter_context(tc.tile_pool(name="ps_e", bufs=2, space="PSUM"))
        ps_big = ctx.enter_context(tc.tile_pool(name="ps_big", bufs=1, space="PSUM"))
        ps_sm = ctx.enter_context(tc.tile_pool(name="ps_sm", bufs=2, space="PSUM"))

        # ---- constants -------------------------------------------------
        ident = pc.tile([P, P], f32, tag="ident")
        make_identity(nc, ident)
        ident_b = pc.tile([P, P], bf, tag="ident_b")
        nc.vector.tensor_copy(ident_b, ident)
        ones_col = pc.tile([P, 1], bf, tag="ones_col")
        nc.vector.memset(ones_col, 1.0)
        ones_one = pc.tile([1, P], bf, tag="ones_one")
        nc.vector.memset(ones_one, 1.0)
        zero_col = pc.tile([P, 1], f32, tag="zero_col")
        nc.vector.memset(zero_col, 0.0)
        eps_col = pc.tile([P, 1], f32, tag="eps_col")
        nc.vector.memset(eps_col, LN_EPS)
        nc.const_aps.aps[(f32, 0.0)] = zero_col
        nc.const_aps.aps[(f32, LN_EPS)] = eps_col
        # SEL[p, c*128+m] = (p == c): replicates row c of an [8,128] rhs
        # across all 128 output partitions via matmul.
        sel = pc.tile([NRC, NRC * P], bf, tag="sel")
        nc.gpsimd.memset(sel, 0.0)
        nc.gpsimd.affine_select(
            out=sel.rearrange("p (c m) -> p c m", m=P),
            in_=sel.rearrange("p (c m) -> p c m", m=P),
            compare_op=mybir.AluOpType.not_equal,
            fill=1.0, base=0,
            pattern=[[-1, NRC], [0, P]],
            channel_multiplier=1,
        )

        def col_vec(dvec, tag):
            v4 = pc.tile([NCC, P], f32, tag=tag + "4")
            nc.sync.dma_start(v4, dvec.rearrange("(c p) -> c p", p=P))
            pst = ps_sm.tile([P, DV], f32, tag="sm", name="pst")
            nc.tensor.transpose(pst[:, :NCC], v4, ident[:NCC, :NCC])
            col = pc.tile([P, NCC], f32, tag=tag + "c")
            nc.vector.tensor_copy(col, pst[:, :NCC])
            return col

        g0_col = col_vec(dg0, "g0")
        b0_col = col_vec(db0, "b0")
        bo_col = col_vec(dbo, "bo")

        def row_vec(dvec, tag):
            rowf = pc.tile([1, DV], f32, tag=tag + "rf")
            nc.sync.dma_start(rowf, dvec[None])
            row = pc.tile([1, DV], bf, tag=tag + "r")
            nc.vector.tensor_copy(row, rowf)
            return row

        # replicated rows [128, DV] via K=1 ones matmul
        def repl_vec(dvec, tag, dt):
            row = row_vec(dvec, tag)
            ps = ps_big.tile([P, N], f32, tag="big", name="repl_ps")
            nc.tensor.matmul(ps[:, :DV], ones_one, row, start=True, stop=True)
            rep = pc.tile([P, DV], dt, tag=tag + "rep")
            nc.vector.tensor_copy(rep, ps[:, :DV])
            return rep

        g1_rep = repl_vec(dg1, "g1v", bf)
        b1_rep = repl_vec(db1, "b1v", f32)
        # bq replicated twice along free: [128, 1024] bf
        bq_row = row_vec(dbq, "bqv")
        bq_ps = ps_big.tile([P, N], f32, tag="big", name="bq_ps")
        nc.tensor.matmul(bq_ps[:, :DV], ones_one, bq_row, start=True, stop=True)
        nc.tensor.matmul(bq_ps[:, DV:], ones_one, bq_row, start=True, stop=True)
        bq_rep2 = pc.tile([P, N], bf, tag="bq_rep2")
        nc.vector.tensor_copy(bq_rep2, bq_ps)

        for _rep in range(reps):
            # ---- load Wq + Q, cast bf16, transpose via xbar DMA ------------
            WqT = [pw.tile([P, DV], bf, tag="wqt", name="wqt") for _ in range(NCC)]
            WoT = [pw.tile([P, DV], bf, tag="wot", name="wot") for _ in range(NCC)]
            QT = [pqt.tile([P, N], bf, tag="qt", name="qt") for _ in range(NCC)]

            for rc in range(NCC):
                wsb = pq.tile([P, DQ], f32, tag="qsb", name="wsb")
                nc.sync.dma_start(wsb, dWq[rc * P:(rc + 1) * P, :])
                pst = ps_sm.tile([P, DV], f32, tag="sm", name="pst")
                for kc in range(NCC):
                    nc.tensor.transpose(pst[:, kc * P:(kc + 1) * P],
                                        wsb[:, kc * P:(kc + 1) * P], ident)
                for kc in range(2):
                    nc.vector.tensor_copy(WqT[2 * kc][:, rc * P:(rc + 1) * P],
                                          pst[:, 2 * kc * P:(2 * kc + 1) * P])
                    nc.scalar.activation(
                        WqT[2 * kc + 1][:, rc * P:(rc + 1) * P],
                        pst[:, (2 * kc + 1) * P:(2 * kc + 2) * P], AF.Copy)

            for rc in range(NRC):
                qsb = pq.tile([P, DQ], f32, tag="qsb", name="qsb")
                nc.sync.dma_start(qsb, dQ[rc * P:(rc + 1) * P, :])
                pst = ps_sm.tile([P, DV], f32, tag="sm", name="pst")
                for kc in range(NCC):
                    nc.tensor.transpose(pst[:, kc * P:(kc + 1) * P],
                                        qsb[:, kc * P:(kc + 1) * P], ident)
                for kc in range(2):
                    nc.vector.tensor_copy(QT[2 * kc][:, rc * P:(rc + 1) * P],
                                          pst[:, 2 * kc * P:(2 * kc + 1) * P])
                    nc.scalar.activation(
                        QT[2 * kc + 1][:, rc * P:(rc + 1) * P],
                        pst[:, (2 * kc + 1) * P:(2 * kc + 2) * P], AF.Copy)

            # ---- Qp = Q@Wq.T + bq (bf, 4 row-pair tiles [128, 1024]) -------
            Qp = [pqp.tile([P, N], bf, tag="qp", name="qp") for _ in range(NCC)]
            for pr in range(NCC):
                ps = ps_e.tile([P, N], f32, tag="eps", name="qp_ps")
                for kc in range(NCC):
                    for half in range(2):
                        rc = 2 * pr + half
                        sl = slice(half * DV, (half + 1) * DV)
                        nc.tensor.matmul(ps[:, sl],
                                         QT[kc][:, rc * P:(rc + 1) * P], WqT[kc],
                                         start=(kc == 0), stop=(kc == NCC - 1))
                nc.vector.tensor_tensor(Qp[pr], ps, bq_rep2, OP.add)

            def qp_slice(jc, lo, hi):
                return Qp[jc // 2][:, (jc % 2) * DV + lo:(jc % 2) * DV + hi]

            # ---- QpT via xbar DMA-transpose --------------------------------
            QpTb = [pqptb.tile([P, N], bf, tag="qptb", name="qptb")
                    for _ in range(NCC)]
            for cc in range(NCC):
                for g in range(2):
                    pst = ps_sm.tile([P, DV], bf, tag="sm", name="pstb")
                    for i in range(4):
                        rc = 4 * g + i
                        nc.tensor.transpose(
                            pst[:, i * P:(i + 1) * P],
                            qp_slice(rc, cc * P, (cc + 1) * P), ident_b)
                    nc.vector.tensor_copy(
                        QpTb[cc][:, g * DV:(g + 1) * DV], pst)

            # ---- Wo loads (needed only by FFN) -----------------------------
            for rc in range(NCC):
                wsb = pq.tile([P, DQ], f32, tag="qsb", name="wsb2")
                nc.sync.dma_start(wsb, dWo[rc * P:(rc + 1) * P, :])
                pst = ps_sm.tile([P, DV], f32, tag="sm", name="pst")
                for kc in range(NCC):
                    nc.tensor.transpose(pst[:, kc * P:(kc + 1) * P],
                                        wsb[:, kc * P:(kc + 1) * P], ident)
                for kc in range(NCC):
                    nc.scalar.activation(WoT[kc][:, rc * P:(rc + 1) * P],
                                         pst[:, kc * P:(kc + 1) * P], AF.Copy)

            # ---- OT accumulator (transposed head outputs + residual) -------
            OT = [pot.tile([P, N], bf, tag="ot", name="ot") for _ in range(NCC)]

            # ---- per-head Sinkhorn attention (software-pipelined) ----------
            e_tiles = {}   # h -> (E chunks, r_mat)

            def emit_E(h, lo, hi):
                tb, po = h // 2, (h % 2) * D
                qht = QpTb[tb][po:po + D, :]
                if h not in e_tiles:
                    E = [pE.tile([P, N], bf, tag="E", name="E") for _ in range(NRC)]
                    r_mat = psm.tile([P, NRC], f32, tag="r_mat", name="r_mat")
                    e_tiles[h] = (E, r_mat)
                E, r_mat = e_tiles[h]
                for ci in range(lo, hi):
                    ps = ps_e.tile([P, N], f32, tag="eps", name="e_ps")
                    for hf in range(2):
                        nc.tensor.matmul(ps[:, hf * DV:(hf + 1) * DV],
                                         qht[:, ci * P:(ci + 1) * P],
                                         qht[:, hf * DV:(hf + 1) * DV],
                                         start=True, stop=True)
                    nc.scalar.activation(E[ci], ps, AF.Exp, scale=SCALE_S,
                                         accum_out=r_mat[:, ci:ci + 1])

            emit_E(0, 0, NRC)
            for h in range(H):
                tb, po = h // 2, (h % 2) * D
                E, r_mat = e_tiles.pop(h)

                if h + 1 < H:
                    emit_E(h + 1, 0, 4)

                # invr (partition) -> invr_rep [128, 1024] bf (free, replicated)
                invr = psm.tile([P, NRC], f32, tag="invr")
                nc.vector.reciprocal(invr, r_mat)
                invr_b = psm.tile([P, NRC], bf, tag="invr_b")
                nc.vector.tensor_copy(invr_b, invr)
                pfl = ps_sm.tile([NRC, P], bf, tag="sm", name="pfl")
                nc.tensor.transpose(pfl, invr_b, ident_b)
                sbt = psm.tile([NRC, P], bf, tag="sbt", bufs=8)
                nc.vector.tensor_copy(sbt, pfl)
                rep_ps = ps_big.tile([P, N], f32, tag="big", name="rep_ps")
                for c in range(NRC):
                    nc.tensor.matmul(rep_ps[:, c * P:(c + 1) * P],
                                     sel[:, c * P:(c + 1) * P], sbt,
                                     start=True, stop=True)
                invr_rep = prep.tile([P, N], bf, tag="irep", name="invr_rep")
                nc.scalar.activation(invr_rep[:, :DV], rep_ps[:, :DV], AF.Copy)
                nc.vector.tensor_copy(invr_rep[:, DV:], rep_ps[:, DV:])

                if h + 1 < H:
                    emit_E(h + 1, 4, NRC)

                # E[jc] *= invr_rep in place (so A@Q absorbs invr);
                # accum gives c_jc per chunk. Pipeline: recip + Qc + A@Q
                # accumulation step chase each chunk.
                c_mat = psm.tile([P, NRC], f32, tag="c_mat")
                invc = psm.tile([P, NRC], f32, tag="invc")
                Qc = [psm.tile([P, D], bf, tag=f"qc{jc}", name=f"qc{jc}")
                      for jc in range(NRC)]
                aq = ps_big.tile([P, N], f32, tag="big", name="aq_ps")
                for jc in range(NRC):
                    nc.vector.scalar_tensor_tensor(
                        E[jc], E[jc], 1.0, invr_rep, OP.mult, OP.mult,
                        accum_out=c_mat[:, jc:jc + 1])
                    nc.vector.reciprocal(invc[:, jc:jc + 1],
                                         c_mat[:, jc:jc + 1])
                    nc.vector.tensor_scalar(Qc[jc],
                                            qp_slice(jc, h * D, (h + 1) * D),
                                            invc[:, jc:jc + 1], AFACT,
                                            OP.mult, OP.mult)
                    for hf in range(2):
                        nc.tensor.matmul(aq[po:po + D, hf * DV:(hf + 1) * DV],
                                         Qc[jc], E[jc][:, hf * DV:(hf + 1) * DV],
                                         start=(jc == 0), stop=(jc == NRC - 1))
                nc.vector.tensor_tensor(OT[tb][po:po + D, :], aq[po:po + D, :],
                                        QpTb[tb][po:po + D, :], OP.add)

            # ---- LN0: per-token stats over features (M=1 ones-matvecs) -----
            sums = ps_e.tile([P, N], f32, tag="eps", name="ln0_sums")
            sqs = ps_e.tile([P, N], f32, tag="eps", name="ln0_sqs")
            for tb in range(NCC):
                sqh = pE.tile([P, N], bf, tag="E", name="sqh")
                nc.vector.tensor_tensor(sqh, OT[tb], OT[tb], OP.mult)
                for hf in range(2):
                    sl = slice(hf * DV, (hf + 1) * DV)
                    nc.tensor.matmul(sums[:1, sl], ones_col, OT[tb][:, sl],
                                     start=(tb == 0), stop=(tb == NCC - 1))
                    nc.tensor.matmul(sqs[:1, sl], ones_col, sqh[:, sl],
                                     start=(tb == 0), stop=(tb == NCC - 1))
            sum_row = psm.tile([1, N], f32, tag="sum_row")
            nc.vector.tensor_copy(sum_row, sums[:1, :])
            sq_row = psm.tile([1, N], f32, tag="sq_row")
            nc.scalar.activation(sq_row, sqs[:1, :], AF.Copy)
            s8 = psm.tile([NRC, P], f32, tag="s8")
            nc.sync.dma_start(s8, sum_row.rearrange("o (a b) -> o a b", a=NRC))
            q8 = psm.tile([NRC, P], f32, tag="q8")
            nc.sync.dma_start(q8, sq_row.rearrange("o (a b) -> o a b", a=NRC))
            pfs = ps_sm.tile([P, 2 * NRC], f32, tag="sm", name="pfs")
            nc.tensor.transpose(pfs[:, :NRC], s8, ident[:NRC, :NRC])
            nc.tensor.transpose(pfs[:, NRC:], q8, ident[:NRC, :NRC])
            m8 = psm.tile([P, NRC], f32, tag="m8")
            nc.vector.tensor_scalar_mul(m8, pfs[:, :NRC], 1.0 / DV)
            v8 = psm.tile([P, NRC], f32, tag="v8")
            nc.vector.tensor_scalar_mul(v8, pfs[:, NRC:], 1.0 / DV)
            t8 = psm.tile([P, NRC], f32, tag="t8")
            nc.vector.tensor_tensor(t8, m8, m8, OP.mult)
            nc.vector.tensor_tensor(v8, v8, t8, OP.subtract)
            sd8 = psm.tile([P, NRC], f32, tag="sd8")
            nc.scalar.activation(sd8, v8, AF.Sqrt, bias=LN_EPS)
            rstd8 = psm.tile([P, NRC], f32, tag="rstd8")
            nc.vector.reciprocal(rstd8, sd8)
            mr8 = psm.tile([P, NRC], bf, tag="mr8")
            nc.vector.tensor_tensor(mr8, m8, rstd8, OP.mult)
            rstd8_b = psm.tile([P, NRC], bf, tag="rstd8b")
            nc.vector.tensor_copy(rstd8_b, rstd8)
            # replicate rstd and m*rstd to [128, 1024] bf (flip + K=1 matmul)
            reps_sb = []
            for idx, src in enumerate((rstd8_b, mr8)):
                pfl2 = ps_sm.tile([NRC, P], bf, tag="sm", name="pfl2")
                nc.tensor.transpose(pfl2, src, ident_b)
                sbt2 = psm.tile([NRC, P], bf, tag=f"sbt{idx}")
                nc.vector.tensor_copy(sbt2, pfl2)
                row2 = psm.tile([1, N], bf, tag=f"row{idx}")
                nc.sync.dma_start(
                    row2.rearrange("o (a b) -> o a b", a=NRC), sbt2)
                rps = ps_big.tile([P, N], f32, tag="big", name="rps")
                nc.tensor.matmul(rps[:, :DV], ones_one, row2[:, :DV],
                                 start=True, stop=True)
                nc.tensor.matmul(rps[:, DV:], ones_one, row2[:, DV:],
                                 start=True, stop=True)
                rep = prep.tile([P, N], bf, tag="irep", name=f"rep{idx}")
                nc.scalar.activation(rep[:, :DV], rps[:, :DV], AF.Copy)
                nc.vector.tensor_copy(rep[:, DV:], rps[:, DV:])
                reps_sb.append(rep)
            rstd_rep, mr_rep = reps_sb

            # normalize: O1T = (OT*rstd - m*rstd)*g0 + b0   (g0/b0 on parts)
            O1T = [po1.tile([P, N], bf, tag="o1t", name="o1t")
                   for _ in range(NCC)]
            for cc in range(NCC):
                t = pE.tile([P, N], bf, tag="E", name="ln0t")
                nc.vector.tensor_tensor(t, OT[cc], rstd_rep, OP.mult)
                nc.vector.tensor_tensor(t, t, mr_rep, OP.subtract)
                nc.vector.tensor_scalar(O1T[cc], t, g0_col[:, cc:cc + 1],
                                        b0_col[:, cc:cc + 1], OP.mult, OP.add)

            # ---- FFN: O2T = O1T + relu(Wo@O1T + bo) ------------------------
            O2T = [po2.tile([P, N], bf, tag="o2t", name="o2t")
                   for _ in range(NCC)]
            for c2 in range(NCC):
                ps = ps_e.tile([P, N], f32, tag="eps", name="ffn_ps")
                for cc in range(NCC):
                    for hf in range(2):
                        nc.tensor.matmul(ps[:, hf * DV:(hf + 1) * DV],
                                         WoT[cc][:, c2 * P:(c2 + 1) * P],
                                         O1T[cc][:, hf * DV:(hf + 1) * DV],
                                         start=(cc == 0), stop=(cc == NCC - 1))
                t = pq.tile([P, N], bf, tag="ffn_t", name="ffn_t")
                nc.scalar.activation(t, ps, AF.Relu, bias=bo_col[:, c2:c2 + 1])
                nc.vector.tensor_tensor(O2T[c2], t, O1T[c2], OP.add)

            # ---- LN1 (row-major after xbar DMA-transpose) + store ----------
            for rc in range(NRC):
                psf = ps_sm.tile([P, DV], bf, tag="sm", name="ln1_psf")
                for cc in range(NCC):
                    nc.tensor.transpose(psf[:, cc * P:(cc + 1) * P],
                                        O2T[cc][:, rc * P:(rc + 1) * P],
                                        ident_b)
                mean = psm.tile([P, 1], f32, tag="ln1_mean")
                nc.vector.tensor_reduce(mean, psf, AX.X, OP.add)
                nc.vector.tensor_scalar_mul(mean, mean, 1.0 / DV)
                xc = pout.tile([P, DV], bf, tag="ln1_xc")
                nc.vector.tensor_scalar_sub(xc, psf, mean)
                sqj = pq.tile([P, DV], bf, tag="ffn_t", name="ln1_sqj")
                ss = psm.tile([P, 1], f32, tag="ln1_ss")
                nc.vector.scalar_tensor_tensor(sqj, xc, 1.0, xc,
                                               OP.mult, OP.mult, accum_out=ss)
                nc.vector.tensor_scalar_mul(ss, ss, 1.0 / DV)
                sd = psm.tile([P, 1], f32, tag="ln1_sd")
                nc.scalar.activation(sd, ss, AF.Sqrt, bias=LN_EPS)
                rstd = psm.tile([P, 1], f32, tag="ln1_rstd")
                nc.vector.reciprocal(rstd, sd)
                ob = pout.tile([P, DV], f32, tag="ln1_out")
                nc.vector.scalar_tensor_tensor(ob, xc, rstd, g1_rep,
                                               OP.mult, OP.mult)
                nc.gpsimd.tensor_tensor(ob, ob, b1_rep, OP.add)
                nc.sync.dma_start(dout[rc * P:(rc + 1) * P, :], ob)

    nc.finalize()
    return nc


def kernel(**inputs):
    from concourse.bass_utils import run_bass_kernel_spmd

    if "nc" not in _CACHE:
        _CACHE["nc"] = _build()
    nc = _CACHE["nc"]

    Q = np.ascontiguousarray(np.asarray(inputs["Q"], dtype=np.float32))
    shared = {k: np.ascontiguousarray(np.asarray(inputs[k], dtype=np.float32))
              for k in ("Wq", "bq", "Wo", "bo", "g0", "b0", "g1", "b1")}
    in_maps = [dict(Q=np.ascontiguousarray(Q[b]), **shared) for b in range(B)]

    res = run_bass_kernel_spmd(nc, in_maps, core_ids=list(range(B)),
                               **_CACHE.get("run_kwargs", {}))
    _CACHE["last_result"] = res
    return np.stack([r["out"] for r in res.results], axis=0)


# revision 11
# speedup vs baseline: 1.5989x; 1.0006x over previous
# kernel.py — MABSINK (Sinkhorn attention block) Trainium2 Bass kernel, v3.
# Self-contained: hardcodes shapes B=8, n=1024, dQ=dV=512, H=8; shards batch
# across 8 NeuronCores (1 batch element per core), runs SPMD, gathers output.
#
# Math (per core, per head h; Q_h = (Q @ Wq.T + bq)[:, h*64:(h+1)*64]):
#   S   = Q_h Q_h^T / sqrt(512)            (symmetric!)
#   E   = exp(S);  r_i = sum_j E_ij;  c_i = sum_n E_in * invr_n (by symmetry)
#   A   = n*mu' * E_ij * invr_i * invc_j,  mu' = 1/n + 1e-8
#   O_h = Q_h + A @ Q_h
# then head-recombine -> LN0 -> x + relu(x@Wo.T+bo) -> LN1.
#
# v3: all input/intermediate transposes via xbar DMA-transpose (bf16),
# freeing the PE; c via DVE tensor_tensor_reduce against invr_rep (no PE
# matvec, no layout flip); invr_rep via [8,128]->[1,1024] DMA reshape +
# K=1 ones matmuls; accumulation chains interleave PSUM banks; head loop
# software-pipelined (E of head h+1 emitted around the A@Q of head h).

import math
import numpy as np

B, N, DQ, DV, H = 8, 1024, 512, 512, 8
D = DV // H          # 64 head dim
P = 128
NRC = N // P         # 8 row chunks
NCC = DV // P        # 4 feature chunks
LN_EPS = 1e-5
SCALE_S = 1.0 / math.sqrt(DV)
AFACT = N * (1.0 / N + 1e-8)   # n * mu'

_CACHE = {}


def _build(reps=1):
    import concourse.mybir as mybir
    from concourse import bacc
    import concourse.tile as tile
    from concourse.masks import make_identity
    from contextlib import ExitStack

    f32 = mybir.dt.float32
    bf = mybir.dt.bfloat16
    AF = mybir.ActivationFunctionType
    OP = mybir.AluOpType
    AX = mybir.AxisListType

    nc = bacc.Bacc()
    dQ = nc.dram_tensor("Q", [N, DQ], f32, kind="ExternalInput")
    dWq = nc.dram_tensor("Wq", [DV, DQ], f32, kind="ExternalInput")
    dbq = nc.dram_tensor("bq", [DQ], f32, kind="ExternalInput")
    dWo = nc.dram_tensor("Wo", [DV, DV], f32, kind="ExternalInput")
    dbo = nc.dram_tensor("bo", [DV], f32, kind="ExternalInput")
    dg0 = nc.dram_tensor("g0", [DV], f32, kind="ExternalInput")
    db0 = nc.dram_tensor("b0", [DV], f32, kind="ExternalInput")
    dg1 = nc.dram_tensor("g1", [DV], f32, kind="ExternalInput")
    db1 = nc.dram_tensor("b1", [DV], f32, kind="ExternalInput")
    dout = nc.dram_tensor("out", [N, DV], f32, kind="ExternalOutput")

    with tile.TileContext(nc) as tc, ExitStack() as ctx:
        pc = ctx.enter_context(tc.tile_pool(name="pc", bufs=1))
        pq = ctx.enter_context(tc.tile_pool(name="pq", bufs=2))
        pqt = ctx.enter_context(tc.tile_pool(name="pqt", bufs=4))
        pw = ctx.enter_context(tc.tile_pool(name="pw", bufs=4))
        pqp = ctx.enter_context(tc.tile_pool(name="pqp", bufs=4))
        pqptb = ctx.enter_context(tc.tile_pool(name="pqptb", bufs=4))
        pE = ctx.enter_context(tc.tile_pool(name="pE", bufs=18))
        pot = ctx.enter_context(tc.tile_pool(name="pot", bufs=4))
        po1 = ctx.enter_context(tc.tile_pool(name="po1", bufs=4))
        po2 = ctx.enter_context(tc.tile_pool(name="po2", bufs=4))
        psm = ctx.enter_context(tc.tile_pool(name="psm", bufs=2))
        prep = ctx.enter_context(tc.tile_pool(name="prep", bufs=2))
        pout = ctx.enter_context(tc.tile_pool(name="pout", bufs=2))

        # PSUM: eps 2x[128,1024]f32 (4 banks) + big 1x[128,1024]f32 (2)
        #       + sm 2x<=[128,512]f32 (2)  = 8 banks
        ps_e = ctx.enter_context(tc.tile_pool(name="ps_e", bufs=2, space="PSUM"))
        ps_big = ctx.enter_context(tc.tile_pool(name="ps_big", bufs=1, space="PSUM"))
        ps_sm = ctx.enter_context(tc.tile_pool(name="ps_sm", bufs=2, space="PSUM"))

        # ---- constants -------------------------------------------------
        ident = pc.tile([P, P], f32, tag="ident")
        make_identity(nc, ident)
        ident_b = pc.tile([P, P], bf, tag="ident_b")
        nc.vector.tensor_copy(ident_b, ident)
        ones_col = pc.tile([P, 1], bf, tag="ones_col")
        nc.vector.memset(ones_col, 1.0)
        ones_one = pc.tile([1, P], bf, tag="ones_one")
        nc.vector.memset(ones_one, 1.0)
        zero_col = pc.tile([P, 1], f32, tag="zero_col")
        nc.vector.memset(zero_col, 0.0)
        eps_col = pc.tile([P, 1], f32, tag="eps_col")
        nc.vector.memset(eps_col, LN_EPS)
        nc.const_aps.aps[(f32, 0.0)] = zero_col
        nc.const_aps.aps[(f32, LN_EPS)] = eps_col
        # SEL[p, c*128+m] = (p == c): replicates row c of an [8,128] rhs
        # across all 128 output partitions via matmul.
        sel = pc.tile([NRC, NRC * P], bf, tag="sel")
        nc.gpsimd.memset(sel, 0.0)
        nc.gpsimd.affine_select(
            out=sel.rearrange("p (c m) -> p c m", m=P),
            in_=sel.rearrange("p (c m) -> p c m", m=P),
            compare_op=mybir.AluOpType.not_equal,
            fill=1.0, base=0,
            pattern=[[-1, NRC], [0, P]],
            channel_multiplier=1,
        )

        def col_vec(dvec, tag):
            v4 = pc.tile([NCC, P], f32, tag=tag + "4")
            nc.sync.dma_start(v4, dvec.rearrange("(c p) -> c p", p=P))
            pst = ps_sm.tile([P, DV], f32, tag="sm", name="pst")
            nc.tensor.transpose(pst[:, :NCC], v4, ident[:NCC, :NCC])
            col = pc.tile([P, NCC], f32, tag=tag + "c")
            nc.vector.tensor_copy(col, pst[:, :NCC])
            return col

        g0_col = col_vec(dg0, "g0")
        b0_col = col_vec(db0, "b0")
        bo_col = col_vec(dbo, "bo")

        def row_vec(dvec, tag):
            rowf = pc.tile([1, DV], f32, tag=tag + "rf")
            nc.sync.dma_start(rowf, dvec[None])
            row = pc.tile([1, DV], bf, tag=tag + "r")
            nc.vector.tensor_copy(row, rowf)
            return row

        # replicated rows [128, DV] via K=1 ones matmul
        def repl_vec(dvec, tag, dt):
            row = row_vec(dvec, tag)
            ps = ps_big.tile([P, N], f32, tag="big", name="repl_ps")
            nc.tensor.matmul(ps[:, :DV], ones_one, row, start=True, stop=True)
            rep = pc.tile([P, DV], dt, tag=tag + "rep")
            nc.vector.tensor_copy(rep, ps[:, :DV])
            return rep

        g1_rep = repl_vec(dg1, "g1v", bf)
        b1_rep = repl_vec(db1, "b1v", f32)
        # bq replicated twice along free: [128, 1024] bf
        bq_row = row_vec(dbq, "bqv")
        bq_ps = ps_big.tile([P, N], f32, tag="big", name="bq_ps")
        nc.tensor.matmul(bq_ps[:, :DV], ones_one, bq_row, start=True, stop=True)
        nc.tensor.matmul(bq_ps[:, DV:], ones_one, bq_row, start=True, stop=True)
        bq_rep2 = pc.tile([P, N], bf, tag="bq_rep2")
        nc.vector.tensor_copy(bq_rep2, bq_ps)

        for _rep in range(reps):
            # ---- load Wq + Q, cast bf16, transpose via xbar DMA ------------
            WqT = [pw.tile([P, DV], bf, tag="wqt", name="wqt") for _ in range(NCC)]
            WoT = [pw.tile([P, DV], bf, tag="wot", name="wot") for _ in range(NCC)]
            QT = [pqt.tile([P, N], bf, tag="qt", name="qt") for _ in range(NCC)]

            for rc in range(NCC):
                wsb = pq.tile([P, DQ], f32, tag="qsb", name="wsb")
                nc.sync.dma_start(wsb, dWq[rc * P:(rc + 1) * P, :])
                pst = ps_sm.tile([P, DV], f32, tag="sm", name="pst")
                for kc in range(NCC):
                    nc.tensor.transpose(pst[:, kc * P:(kc + 1) * P],
                                        wsb[:, kc * P:(kc + 1) * P], ident)
                for kc in range(NCC):
                    nc.scalar.activation(
                        WqT[kc][:, rc * P:(rc + 1) * P],
                        pst[:, kc * P:(kc + 1) * P], AF.Copy)

            for rc in range(NRC):
                qsb = pq.tile([P, DQ], f32, tag="qsb", name="qsb")
                nc.sync.dma_start(qsb, dQ[rc * P:(rc + 1) * P, :])
                pst = ps_sm.tile([P, DV], f32, tag="sm", name="pst")
                for kc in range(NCC):
                    nc.tensor.transpose(pst[:, kc * P:(kc + 1) * P],
                                        qsb[:, kc * P:(kc + 1) * P], ident)
                for kc in range(2):
                    nc.vector.tensor_copy(QT[2 * kc][:, rc * P:(rc + 1) * P],
                                          pst[:, 2 * kc * P:(2 * kc + 1) * P])
                    nc.scalar.activation(
                        QT[2 * kc + 1][:, rc * P:(rc + 1) * P],
                        pst[:, (2 * kc + 1) * P:(2 * kc + 2) * P], AF.Copy)

            # ---- Qp = Q@Wq.T + bq (bf, 4 row-pair tiles [128, 1024]) -------
            Qp = [pqp.tile([P, N], bf, tag="qp", name="qp") for _ in range(NCC)]
            for pr in range(NCC):
                ps = ps_e.tile([P, N], f32, tag="eps", name="qp_ps")
                for kc in range(NCC):
                    for half in range(2):
                        rc = 2 * pr + half
                        sl = slice(half * DV, (half + 1) * DV)
                        nc.tensor.matmul(ps[:, sl],
                                         QT[kc][:, rc * P:(rc + 1) * P], WqT[kc],
                                         start=(kc == 0), stop=(kc == NCC - 1))
                nc.vector.tensor_tensor(Qp[pr], ps, bq_rep2, OP.add)

            def qp_slice(jc, lo, hi):
                return Qp[jc // 2][:, (jc % 2) * DV + lo:(jc % 2) * DV + hi]

            # ---- QpT via xbar DMA-transpose --------------------------------
            QpTb = [pqptb.tile([P, N], bf, tag="qptb", name="qptb")
                    for _ in range(NCC)]
            for cc in range(NCC):
                for g in range(2):
                    pst = ps_sm.tile([P, DV], bf, tag="sm", name="pstb")
                    for i in range(4):
                        rc = 4 * g + i
                        nc.tensor.transpose(
                            pst[:, i * P:(i + 1) * P],
                            qp_slice(rc, cc * P, (cc + 1) * P), ident_b)
                    nc.vector.tensor_copy(
                        QpTb[cc][:, g * DV:(g + 1) * DV], pst)

            # ---- Wo loads (needed only by FFN) -----------------------------
            for rc in range(NCC):
                wsb = pq.tile([P, DQ], f32, tag="qsb", name="wsb2")
                nc.sync.dma_start(wsb, dWo[rc * P:(rc + 1) * P, :])
                pst = ps_sm.tile([P, DV], f32, tag="sm", name="pst")
                for kc in range(NCC):
                    nc.tensor.transpose(pst[:, kc * P:(kc + 1) * P],
                                        wsb[:, kc * P:(kc + 1) * P], ident)
                for kc in range(NCC):
                    nc.scalar.activation(WoT[kc][:, rc * P:(rc + 1) * P],
                                         pst[:, kc * P:(kc + 1) * P], AF.Copy)

            # ---- OT accumulator (transposed head outputs + residual) -------
            OT = [pot.tile([P, N], bf, tag="ot", name="ot") for _ in range(NCC)]

            # ---- per-head Sinkhorn attention (software-pipelined) ----------
            e_tiles = {}   # h -> (E chunks, r_mat)

            def emit_E(h, lo, hi):
                tb, po = h // 2, (h % 2) * D
                qht = QpTb[tb][po:po + D, :]
                if h not in e_tiles:
                    E = [pE.tile([P, N], bf, tag="E", name="E") for _ in range(NRC)]
                    r_mat = psm.tile([P, NRC], f32, tag="r_mat", name="r_mat")
                    e_tiles[h] = (E, r_mat)
                E, r_mat = e_tiles[h]
                for ci in range(lo, hi):
                    ps = ps_e.tile([P, N], f32, tag="eps", name="e_ps")
                    for hf in range(2):
                        nc.tensor.matmul(ps[:, hf * DV:(hf + 1) * DV],
                                         qht[:, ci * P:(ci + 1) * P],
                                         qht[:, hf * DV:(hf + 1) * DV],
                                         start=True, stop=True)
                    nc.scalar.activation(E[ci], ps, AF.Exp, scale=SCALE_S,
                                         accum_out=r_mat[:, ci:ci + 1])

            emit_E(0, 0, NRC)
            for h in range(H):
                tb, po = h // 2, (h % 2) * D
                E, r_mat = e_tiles.pop(h)

                if h + 1 < H:
                    emit_E(h + 1, 0, 4)

                # invr (partition) -> invr_rep [128, 1024] bf (free, replicated)
                invr = psm.tile([P, NRC], f32, tag="invr")
                nc.vector.reciprocal(invr, r_mat)
                invr_b = psm.tile([P, NRC], bf, tag="invr_b")
                nc.vector.tensor_copy(invr_b, invr)
                pfl = ps_sm.tile([NRC, P], bf, tag="sm", name="pfl")
                nc.tensor.transpose(pfl, invr_b, ident_b)
                sbt = psm.tile([NRC, P], bf, tag="sbt", bufs=8)
                nc.vector.tensor_copy(sbt, pfl)
                rep_ps = ps_big.tile([P, N], f32, tag="big", name="rep_ps")
                for c in range(NRC):
                    nc.tensor.matmul(rep_ps[:, c * P:(c + 1) * P],
                                     sel[:, c * P:(c + 1) * P], sbt,
                                     start=True, stop=True)
                invr_rep = prep.tile([P, N], bf, tag="irep", name="invr_rep")
                nc.scalar.activation(invr_rep[:, :DV], rep_ps[:, :DV], AF.Copy)
                nc.vector.tensor_copy(invr_rep[:, DV:], rep_ps[:, DV:])

                if h + 1 < H:
                    emit_E(h + 1, 4, NRC)

                # E[jc] *= invr_rep in place (so A@Q absorbs invr);
                # accum gives c_jc per chunk. Pipeline: recip + Qc + A@Q
                # accumulation step chase each chunk.
                c_mat = psm.tile([P, NRC], f32, tag="c_mat")
                invc = psm.tile([P, NRC], f32, tag="invc")
                Qc = [psm.tile([P, D], bf, tag=f"qc{jc}", name=f"qc{jc}")
                      for jc in range(NRC)]
                aq = ps_big.tile([P, N], f32, tag="big", name="aq_ps")
                for jc in range(NRC):
                    nc.vector.scalar_tensor_tensor(
                        E[jc], E[jc], 1.0, invr_rep, OP.mult, OP.mult,
                        accum_out=c_mat[:, jc:jc + 1])
                    nc.vector.reciprocal(invc[:, jc:jc + 1],
                                         c_mat[:, jc:jc + 1])
                    nc.vector.tensor_scalar(Qc[jc],
                                            qp_slice(jc, h * D, (h + 1) * D),
                                            invc[:, jc:jc + 1], AFACT,
                                            OP.mult, OP.mult)
                    for hf in range(2):
                        nc.tensor.matmul(aq[po:po + D, hf * DV:(hf + 1) * DV],
                                         Qc[jc], E[jc][:, hf * DV:(hf + 1) * DV],
                                         start=(jc == 0), stop=(jc == NRC - 1))
                nc.vector.tensor_tensor(OT[tb][po:po + D, :], aq[po:po + D, :],
                                        QpTb[tb][po:po + D, :], OP.add)

            # ---- LN0: per-token stats over features (M=1 ones-matvecs) -----
            sums = ps_e.tile([P, N], f32, tag="eps", name="ln0_sums")
            sqs = ps_e.tile([P, N], f32, tag="eps", name="ln0_sqs")
            for tb in range(NCC):
                sqh = pE.tile([P, N], bf, tag="E", name="sqh")
                nc.vector.tensor_tensor(sqh, OT[tb], OT[tb], OP.mult)
                for hf in range(2):
                    sl = slice(hf * DV, (hf + 1) * DV)
                    nc.tensor.matmul(sums[:1, sl], ones_col, OT[tb][:, sl],
                                     start=(tb == 0), stop=(tb == NCC - 1))
                    nc.tensor.matmul(sqs[:1, sl], ones_col, sqh[:, sl],
                                     start=(tb == 0), stop=(tb == NCC - 1))
            sum_row = psm.tile([1, N], f32, tag="sum_row")
            nc.vector.tensor_copy(sum_row, sums[:1, :])
            sq_row = psm.tile([1, N], f32, tag="sq_row")
            nc.scalar.activation(sq_row, sqs[:1, :], AF.Copy)
            s8 = psm.tile([NRC, P], f32, tag="s8")
            nc.sync.dma_start(s8, sum_row.rearrange("o (a b) -> o a b", a=NRC))
            q8 = psm.tile([NRC, P], f32, tag="q8")
            nc.sync.dma_start(q8, sq_row.rearrange("o (a b) -> o a b", a=NRC))
            pfs = ps_sm.tile([P, 2 * NRC], f32, tag="sm", name="pfs")
            nc.tensor.transpose(pfs[:, :NRC], s8, ident[:NRC, :NRC])
            nc.tensor.transpose(pfs[:, NRC:], q8, ident[:NRC, :NRC])
            m8 = psm.tile([P, NRC], f32, tag="m8")
            nc.vector.tensor_scalar_mul(m8, pfs[:, :NRC], 1.0 / DV)
            v8 = psm.tile([P, NRC], f32, tag="v8")
            nc.vector.tensor_scalar_mul(v8, pfs[:, NRC:], 1.0 / DV)
            t8 = psm.tile([P, NRC], f32, tag="t8")
            nc.vector.tensor_tensor(t8, m8, m8, OP.mult)
            nc.vector.tensor_tensor(v8, v8, t8, OP.subtract)
            sd8 = psm.tile([P, NRC], f32, tag="sd8")
            nc.scalar.activation(sd8, v8, AF.Sqrt, bias=LN_EPS)
            rstd8 = psm.tile([P, NRC], f32, tag="rstd8")
            nc.vector.reciprocal(rstd8, sd8)
            mr8 = psm.tile([P, NRC], bf, tag="mr8")
            nc.vector.tensor_tensor(mr8, m8, rstd8, OP.mult)
            rstd8_b = psm.tile([P, NRC], bf, tag="rstd8b")
            nc.vector.tensor_copy(rstd8_b, rstd8)
            # replicate rstd and m*rstd to [128, 1024] bf (flip + K=1 matmul)
            reps_sb = []
            for idx, src in enumerate((rstd8_b, mr8)):
                pfl2 = ps_sm.tile([NRC, P], bf, tag="sm", name="pfl2")
                nc.tensor.transpose(pfl2, src, ident_b)
                sbt2 = psm.tile([NRC, P], bf, tag=f"sbt{idx}")
                nc.vector.tensor_copy(sbt2, pfl2)
                row2 = psm.tile([1, N], bf, tag=f"row{idx}")
                nc.sync.dma_start(
                    row2.rearrange("o (a b) -> o a b", a=NRC), sbt2)
                rps = ps_big.tile([P, N], f32, tag="big", name="rps")
                nc.tensor.matmul(rps[:, :DV], ones_one, row2[:, :DV],
                                 start=True, stop=True)
                nc.tensor.matmul(rps[:, DV:], ones_one, row2[:, DV:],
                                 start=True, stop=True)
                rep = prep.tile([P, N], bf, tag="irep", name=f"rep{idx}")
                nc.scalar.activation(rep[:, :DV], rps[:, :DV], AF.Copy)
                nc.vector.tensor_copy(rep[:, DV:], rps[:, DV:])
                reps_sb.append(rep)
            rstd_rep, mr_rep = reps_sb

            # normalize: O1T = (OT*rstd - m*rstd)*g0 + b0   (g0/b0 on parts)
            O1T = [po1.tile([P, N], bf, tag="o1t", name="o1t")
                   for _ in range(NCC)]
            for cc in range(NCC):
                t = pE.tile([P, N], bf, tag="E", name="ln0t")
                eng = nc.gpsimd if cc % 2 == 0 else nc.vector
                eng.tensor_tensor(t, OT[cc], rstd_rep, OP.mult)
                nc.vector.tensor_tensor(t, t, mr_rep, OP.subtract)
                nc.vector.tensor_scalar(O1T[cc], t, g0_col[:, cc:cc + 1],
                                        b0_col[:, cc:cc + 1], OP.mult, OP.add)

            # ---- FFN: O2T = O1T + relu(Wo@O1T + bo) ------------------------
            O2T = [po2.tile([P, N], bf, tag="o2t", name="o2t")
                   for _ in range(NCC)]
            for c2 in range(NCC):
                ps = ps_e.tile([P, N], f32, tag="eps", name="ffn_ps")
                for cc in range(NCC):
                    for hf in range(2):
                        nc.tensor.matmul(ps[:, hf * DV:(hf + 1) * DV],
                                         WoT[cc][:, c2 * P:(c2 + 1) * P],
                                         O1T[cc][:, hf * DV:(hf + 1) * DV],
                                         start=(cc == 0), stop=(cc == NCC - 1))
                t = pq.tile([P, N], bf, tag="ffn_t", name="ffn_t")
                nc.scalar.activation(t, ps, AF.Relu, bias=bo_col[:, c2:c2 + 1])
                nc.vector.tensor_tensor(O2T[c2], t, O1T[c2], OP.add)

            # ---- LN1 (row-major after xbar DMA-transpose) + store ----------
            for rc in range(NRC):
                psf = ps_sm.tile([P, DV], bf, tag="sm", name="ln1_psf")
                for cc in range(NCC):
                    nc.tensor.transpose(psf[:, cc * P:(cc + 1) * P],
                                        O2T[cc][:, rc * P:(rc + 1) * P],
                                        ident_b)
                mean = psm.tile([P, 1], f32, tag="ln1_mean")
                msc = pq.tile([P, DV], bf, tag="ffn_t", name="ln1_msc")
                nc.scalar.activation(msc, psf, AF.Copy, scale=1.0 / DV,
                                     accum_out=mean)
                xc = pout.tile([P, DV], bf, tag="ln1_xc")
                nc.vector.tensor_scalar_sub(xc, psf, mean)
                sqj = pq.tile([P, DV], bf, tag="ffn_t", name="ln1_sqj")
                ss = psm.tile([P, 1], f32, tag="ln1_ss")
                nc.scalar.activation(sqj, xc, AF.Square, accum_out=ss)
                nc.vector.tensor_scalar_mul(ss, ss, 1.0 / DV)
                sd = psm.tile([P, 1], f32, tag="ln1_sd")
                nc.scalar.activation(sd, ss, AF.Sqrt, bias=LN_EPS)
                rstd = psm.tile([P, 1], f32, tag="ln1_rstd")
                nc.vector.reciprocal(rstd, sd)
                ob = pout.tile([P, DV], f32, tag="ln1_out")
                nc.vector.scalar_tensor_tensor(ob, xc, rstd, g1_rep,
                                               OP.mult, OP.mult)
                nc.gpsimd.tensor_tensor(ob, ob, b1_rep, OP.add)
                nc.sync.dma_start(dout[rc * P:(rc + 1) * P, :], ob)

    nc.finalize()
    return nc


def kernel(**inputs):
    from concourse.bass_utils import run_bass_kernel_spmd

    if "nc" not in _CACHE:
        _CACHE["nc"] = _build()
    nc = _CACHE["nc"]

    Q = np.ascontiguousarray(np.asarray(inputs["Q"], dtype=np.float32))
    shared = {k: np.ascontiguousarray(np.asarray(inputs[k], dtype=np.float32))
              for k in ("Wq", "bq", "Wo", "bo", "g0", "b0", "g1", "b1")}
    in_maps = [dict(Q=np.ascontiguousarray(Q[b]), **shared) for b in range(B)]

    res = run_bass_kernel_spmd(nc, in_maps, core_ids=list(range(B)),
                               **_CACHE.get("run_kwargs", {}))
    _CACHE["last_result"] = res
    return np.stack([r["out"] for r in res.results], axis=0)


# revision 12
# speedup vs baseline: 1.6087x; 1.0062x over previous
# kernel.py — MABSINK (Sinkhorn attention block) Trainium2 Bass kernel, v3.
# Self-contained: hardcodes shapes B=8, n=1024, dQ=dV=512, H=8; shards batch
# across 8 NeuronCores (1 batch element per core), runs SPMD, gathers output.
#
# Math (per core, per head h; Q_h = (Q @ Wq.T + bq)[:, h*64:(h+1)*64]):
#   S   = Q_h Q_h^T / sqrt(512)            (symmetric!)
#   E   = exp(S);  r_i = sum_j E_ij;  c_i = sum_n E_in * invr_n (by symmetry)
#   A   = n*mu' * E_ij * invr_i * invc_j,  mu' = 1/n + 1e-8
#   O_h = Q_h + A @ Q_h
# then head-recombine -> LN0 -> x + relu(x@Wo.T+bo) -> LN1.
#
# v3: all input/intermediate transposes via xbar DMA-transpose (bf16),
# freeing the PE; c via DVE tensor_tensor_reduce against invr_rep (no PE
# matvec, no layout flip); invr_rep via [8,128]->[1,1024] DMA reshape +
# K=1 ones matmuls; accumulation chains interleave PSUM banks; head loop
# software-pipelined (E of head h+1 emitted around the A@Q of head h).

import math
import numpy as np

B, N, DQ, DV, H = 8, 1024, 512, 512, 8
D = DV // H          # 64 head dim
P = 128
NRC = N // P         # 8 row chunks
NCC = DV // P        # 4 feature chunks
LN_EPS = 1e-5
SCALE_S = 1.0 / math.sqrt(DV)
AFACT = N * (1.0 / N + 1e-8)   # n * mu'

_CACHE = {}


def _build(reps=1):
    import concourse.mybir as mybir
    from concourse import bacc
    import concourse.tile as tile
    from concourse.masks import make_identity
    from contextlib import ExitStack

    f32 = mybir.dt.float32
    bf = mybir.dt.bfloat16
    AF = mybir.ActivationFunctionType
    OP = mybir.AluOpType
    AX = mybir.AxisListType

    nc = bacc.Bacc()
    dQ = nc.dram_tensor("Q", [N, DQ], f32, kind="ExternalInput")
    dWq = nc.dram_tensor("Wq", [DV, DQ], f32, kind="ExternalInput")
    dbq = nc.dram_tensor("bq", [DQ], f32, kind="ExternalInput")
    dWo = nc.dram_tensor("Wo", [DV, DV], f32, kind="ExternalInput")
    dbo = nc.dram_tensor("bo", [DV], f32, kind="ExternalInput")
    dg0 = nc.dram_tensor("g0", [DV], f32, kind="ExternalInput")
    db0 = nc.dram_tensor("b0", [DV], f32, kind="ExternalInput")
    dg1 = nc.dram_tensor("g1", [DV], f32, kind="ExternalInput")
    db1 = nc.dram_tensor("b1", [DV], f32, kind="ExternalInput")
    dout = nc.dram_tensor("out", [N, DV], f32, kind="ExternalOutput")

    with tile.TileContext(nc) as tc, ExitStack() as ctx:
        pc = ctx.enter_context(tc.tile_pool(name="pc", bufs=1))
        pq = ctx.enter_context(tc.tile_pool(name="pq", bufs=2))
        pqt = ctx.enter_context(tc.tile_pool(name="pqt", bufs=4))
        pw = ctx.enter_context(tc.tile_pool(name="pw", bufs=4))
        pqp = ctx.enter_context(tc.tile_pool(name="pqp", bufs=4))
        pqptb = ctx.enter_context(tc.tile_pool(name="pqptb", bufs=4))
        pE = ctx.enter_context(tc.tile_pool(name="pE", bufs=18))
        pot = ctx.enter_context(tc.tile_pool(name="pot", bufs=4))
        po1 = ctx.enter_context(tc.tile_pool(name="po1", bufs=4))
        po2 = ctx.enter_context(tc.tile_pool(name="po2", bufs=4))
        psm = ctx.enter_context(tc.tile_pool(name="psm", bufs=2))
        prep = ctx.enter_context(tc.tile_pool(name="prep", bufs=2))
        pout = ctx.enter_context(tc.tile_pool(name="pout", bufs=2))

        # PSUM: eps 2x[128,1024]f32 (4 banks) + big 1x[128,1024]f32 (2)
        #       + sm 2x<=[128,512]f32 (2)  = 8 banks
        ps_e = ctx.enter_context(tc.tile_pool(name="ps_e", bufs=2, space="PSUM"))
        ps_big = ctx.enter_context(tc.tile_pool(name="ps_big", bufs=1, space="PSUM"))
        ps_sm = ctx.enter_context(tc.tile_pool(name="ps_sm", bufs=2, space="PSUM"))

        # ---- constants -------------------------------------------------
        ident = pc.tile([P, P], f32, tag="ident")
        make_identity(nc, ident)
        ident_b = pc.tile([P, P], bf, tag="ident_b")
        nc.vector.tensor_copy(ident_b, ident)
        ones_col = pc.tile([P, 1], bf, tag="ones_col")
        nc.vector.memset(ones_col, 1.0)
        ones_one = pc.tile([1, P], bf, tag="ones_one")
        nc.vector.memset(ones_one, 1.0)
        zero_col = pc.tile([P, 1], f32, tag="zero_col")
        nc.vector.memset(zero_col, 0.0)
        eps_col = pc.tile([P, 1], f32, tag="eps_col")
        nc.vector.memset(eps_col, LN_EPS)
        nc.const_aps.aps[(f32, 0.0)] = zero_col
        nc.const_aps.aps[(f32, LN_EPS)] = eps_col
        # SEL[p, c*128+m] = (p == c): replicates row c of an [8,128] rhs
        # across all 128 output partitions via matmul.
        sel = pc.tile([NRC, NRC * P], bf, tag="sel")
        nc.gpsimd.memset(sel, 0.0)
        nc.gpsimd.affine_select(
            out=sel.rearrange("p (c m) -> p c m", m=P),
            in_=sel.rearrange("p (c m) -> p c m", m=P),
            compare_op=mybir.AluOpType.not_equal,
            fill=1.0, base=0,
            pattern=[[-1, NRC], [0, P]],
            channel_multiplier=1,
        )

        def col_vec(dvec, tag):
            v4 = pc.tile([NCC, P], f32, tag=tag + "4")
            nc.sync.dma_start(v4, dvec.rearrange("(c p) -> c p", p=P))
            pst = ps_sm.tile([P, DV], f32, tag="sm", name="pst")
            nc.tensor.transpose(pst[:, :NCC], v4, ident[:NCC, :NCC])
            col = pc.tile([P, NCC], f32, tag=tag + "c")
            nc.vector.tensor_copy(col, pst[:, :NCC])
            return col

        g0_col = col_vec(dg0, "g0")
        b0_col = col_vec(db0, "b0")
        bo_col = col_vec(dbo, "bo")

        def row_vec(dvec, tag):
            rowf = pc.tile([1, DV], f32, tag=tag + "rf")
            nc.sync.dma_start(rowf, dvec[None])
            row = pc.tile([1, DV], bf, tag=tag + "r")
            nc.vector.tensor_copy(row, rowf)
            return row

        # replicated rows [128, DV] via K=1 ones matmul
        def repl_vec(dvec, tag, dt):
            row = row_vec(dvec, tag)
            ps = ps_big.tile([P, N], f32, tag="big", name="repl_ps")
            nc.tensor.matmul(ps[:, :DV], ones_one, row, start=True, stop=True)
            rep = pc.tile([P, DV], dt, tag=tag + "rep")
            nc.vector.tensor_copy(rep, ps[:, :DV])
            return rep

        g1_rep = repl_vec(dg1, "g1v", bf)
        b1_rep = repl_vec(db1, "b1v", f32)
        # bq replicated twice along free: [128, 1024] bf
        bq_row = row_vec(dbq, "bqv")
        bq_ps = ps_big.tile([P, N], f32, tag="big", name="bq_ps")
        nc.tensor.matmul(bq_ps[:, :DV], ones_one, bq_row, start=True, stop=True)
        nc.tensor.matmul(bq_ps[:, DV:], ones_one, bq_row, start=True, stop=True)
        bq_rep2 = pc.tile([P, N], bf, tag="bq_rep2")
        nc.vector.tensor_copy(bq_rep2, bq_ps)

        for _rep in range(reps):
            # ---- load Wq + Q, cast bf16, transpose via xbar DMA ------------
            WqT = [pw.tile([P, DV], bf, tag="wqt", name="wqt") for _ in range(NCC)]
            WoT = [pw.tile([P, DV], bf, tag="wot", name="wot") for _ in range(NCC)]
            QT = [pqt.tile([P, N], bf, tag="qt", name="qt") for _ in range(NCC)]

            for rc in range(NCC):
                wsb = pq.tile([P, DQ], f32, tag="qsb", name="wsb")
                nc.sync.dma_start(wsb, dWq[rc * P:(rc + 1) * P, :])
                pst = ps_sm.tile([P, DV], f32, tag="sm", name="pst")
                for kc in range(NCC):
                    nc.tensor.transpose(pst[:, kc * P:(kc + 1) * P],
                                        wsb[:, kc * P:(kc + 1) * P], ident)
                for kc in range(NCC):
                    nc.scalar.activation(
                        WqT[kc][:, rc * P:(rc + 1) * P],
                        pst[:, kc * P:(kc + 1) * P], AF.Copy)

            for rc in range(NRC):
                qsb = pq.tile([P, DQ], f32, tag="qsb", name="qsb")
                nc.sync.dma_start(qsb, dQ[rc * P:(rc + 1) * P, :])
                pst = ps_sm.tile([P, DV], f32, tag="sm", name="pst")
                for kc in range(NCC):
                    nc.tensor.transpose(pst[:, kc * P:(kc + 1) * P],
                                        qsb[:, kc * P:(kc + 1) * P], ident)
                for kc in range(2):
                    nc.vector.tensor_copy(QT[2 * kc][:, rc * P:(rc + 1) * P],
                                          pst[:, 2 * kc * P:(2 * kc + 1) * P])
                    nc.scalar.activation(
                        QT[2 * kc + 1][:, rc * P:(rc + 1) * P],
                        pst[:, (2 * kc + 1) * P:(2 * kc + 2) * P], AF.Copy)

            # ---- Qp = Q@Wq.T + bq (bf, 4 row-pair tiles [128, 1024]) -------
            Qp = [pqp.tile([P, N], bf, tag="qp", name="qp") for _ in range(NCC)]
            for pr in range(NCC):
                ps = ps_e.tile([P, N], f32, tag="eps", name="qp_ps")
                for kc in range(NCC):
                    for half in range(2):
                        rc = 2 * pr + half
                        sl = slice(half * DV, (half + 1) * DV)
                        nc.tensor.matmul(ps[:, sl],
                                         QT[kc][:, rc * P:(rc + 1) * P], WqT[kc],
                                         start=(kc == 0), stop=(kc == NCC - 1))
                nc.vector.tensor_tensor(Qp[pr], ps, bq_rep2, OP.add)

            def qp_slice(jc, lo, hi):
                return Qp[jc // 2][:, (jc % 2) * DV + lo:(jc % 2) * DV + hi]

            # ---- QpT via xbar DMA-transpose --------------------------------
            QpTb = [pqptb.tile([P, N], bf, tag="qptb", name="qptb")
                    for _ in range(NCC)]
            for cc in range(NCC):
                for g in range(2):
                    pst = ps_sm.tile([P, DV], bf, tag="sm", name="pstb")
                    for i in range(4):
                        rc = 4 * g + i
                        nc.tensor.transpose(
                            pst[:, i * P:(i + 1) * P],
                            qp_slice(rc, cc * P, (cc + 1) * P), ident_b)
                    nc.vector.tensor_copy(
                        QpTb[cc][:, g * DV:(g + 1) * DV], pst)

            # ---- Wo loads (needed only by FFN) -----------------------------
            for rc in range(NCC):
                wsb = pq.tile([P, DQ], f32, tag="qsb", name="wsb2")
                nc.sync.dma_start(wsb, dWo[rc * P:(rc + 1) * P, :])
                pst = ps_sm.tile([P, DV], f32, tag="sm", name="pst")
                for kc in range(NCC):
                    nc.tensor.transpose(pst[:, kc * P:(kc + 1) * P],
                                        wsb[:, kc * P:(kc + 1) * P], ident)
                for kc in range(NCC):
                    nc.scalar.activation(WoT[kc][:, rc * P:(rc + 1) * P],
                                         pst[:, kc * P:(kc + 1) * P], AF.Copy)

            # ---- OT accumulator (transposed head outputs + residual) -------
            OT = [pot.tile([P, N], bf, tag="ot", name="ot") for _ in range(NCC)]

            # ---- per-head Sinkhorn attention (software-pipelined) ----------
            e_tiles = {}   # h -> (E chunks, r_mat)

            def emit_E(h, lo, hi):
                tb, po = h // 2, (h % 2) * D
                qht = QpTb[tb][po:po + D, :]
                if h not in e_tiles:
                    E = [pE.tile([P, N], bf, tag="E", name="E") for _ in range(NRC)]
                    r_mat = psm.tile([P, NRC], f32, tag="r_mat", name="r_mat")
                    e_tiles[h] = (E, r_mat)
                E, r_mat = e_tiles[h]
                for ci in range(lo, hi):
                    ps = ps_e.tile([P, N], f32, tag="eps", name="e_ps")
                    for hf in range(2):
                        nc.tensor.matmul(ps[:, hf * DV:(hf + 1) * DV],
                                         qht[:, ci * P:(ci + 1) * P],
                                         qht[:, hf * DV:(hf + 1) * DV],
                                         start=True, stop=True)
                    nc.scalar.activation(E[ci], ps, AF.Exp, scale=SCALE_S,
                                         accum_out=r_mat[:, ci:ci + 1])

            emit_E(0, 0, NRC)
            for h in range(H):
                tb, po = h // 2, (h % 2) * D
                E, r_mat = e_tiles.pop(h)

                # invr (partition) -> invr_rep [128, 1024] bf (free, replicated)
                invr = psm.tile([P, NRC], f32, tag="invr")
                nc.vector.reciprocal(invr, r_mat)
                invr_b = psm.tile([P, NRC], bf, tag="invr_b")
                nc.vector.tensor_copy(invr_b, invr)
                pfl = ps_sm.tile([NRC, P], bf, tag="sm", name="pfl")
                nc.tensor.transpose(pfl, invr_b, ident_b)
                sbt = psm.tile([NRC, P], bf, tag="sbt", bufs=8)
                nc.vector.tensor_copy(sbt, pfl)
                rep_ps = ps_big.tile([P, N], f32, tag="big", name="rep_ps")
                for c in range(NRC):
                    nc.tensor.matmul(rep_ps[:, c * P:(c + 1) * P],
                                     sel[:, c * P:(c + 1) * P], sbt,
                                     start=True, stop=True)
                invr_rep = prep.tile([P, N], bf, tag="irep", name="invr_rep")
                nc.scalar.activation(invr_rep[:, :DV], rep_ps[:, :DV], AF.Copy)
                nc.vector.tensor_copy(invr_rep[:, DV:], rep_ps[:, DV:])

                if h + 1 < H:
                    emit_E(h + 1, 0, 4)

                # E[jc] *= invr_rep in place (so A@Q absorbs invr);
                # accum gives c_jc per chunk. Pipeline: recip + Qc + A@Q
                # accumulation step chase each chunk.
                c_mat = psm.tile([P, NRC], f32, tag="c_mat")
                invc = psm.tile([P, NRC], f32, tag="invc")
                Qc = [psm.tile([P, D], bf, tag=f"qc{jc}", name=f"qc{jc}")
                      for jc in range(NRC)]
                aq = ps_big.tile([P, N], f32, tag="big", name="aq_ps")
                for jc in range(NRC):
                    if jc == 4 and h + 1 < H:
                        emit_E(h + 1, 4, NRC)
                    nc.vector.scalar_tensor_tensor(
                        E[jc], E[jc], 1.0, invr_rep, OP.mult, OP.mult,
                        accum_out=c_mat[:, jc:jc + 1])
                    nc.vector.reciprocal(invc[:, jc:jc + 1],
                                         c_mat[:, jc:jc + 1])
                    nc.vector.tensor_scalar(Qc[jc],
                                            qp_slice(jc, h * D, (h + 1) * D),
                                            invc[:, jc:jc + 1], AFACT,
                                            OP.mult, OP.mult)
                    for hf in range(2):
                        nc.tensor.matmul(aq[po:po + D, hf * DV:(hf + 1) * DV],
                                         Qc[jc], E[jc][:, hf * DV:(hf + 1) * DV],
                                         start=(jc == 0), stop=(jc == NRC - 1))
                nc.vector.tensor_tensor(OT[tb][po:po + D, :], aq[po:po + D, :],
                                        QpTb[tb][po:po + D, :], OP.add)

            # ---- LN0: per-token stats over features (M=1 ones-matvecs) -----
            sums = ps_e.tile([P, N], f32, tag="eps", name="ln0_sums")
            sqs = ps_e.tile([P, N], f32, tag="eps", name="ln0_sqs")
            for tb in range(NCC):
                sqh = pE.tile([P, N], bf, tag="E", name="sqh")
                nc.vector.tensor_tensor(sqh, OT[tb], OT[tb], OP.mult)
                for hf in range(2):
                    sl = slice(hf * DV, (hf + 1) * DV)
                    nc.tensor.matmul(sums[:1, sl], ones_col, OT[tb][:, sl],
                                     start=(tb == 0), stop=(tb == NCC - 1))
                    nc.tensor.matmul(sqs[:1, sl], ones_col, sqh[:, sl],
                                     start=(tb == 0), stop=(tb == NCC - 1))
            sum_row = psm.tile([1, N], f32, tag="sum_row")
            nc.vector.tensor_copy(sum_row, sums[:1, :])
            sq_row = psm.tile([1, N], f32, tag="sq_row")
            nc.scalar.activation(sq_row, sqs[:1, :], AF.Copy)
            s8 = psm.tile([NRC, P], f32, tag="s8")
            nc.sync.dma_start(s8, sum_row.rearrange("o (a b) -> o a b", a=NRC))
            q8 = psm.tile([NRC, P], f32, tag="q8")
            nc.sync.dma_start(q8, sq_row.rearrange("o (a b) -> o a b", a=NRC))
            pfs = ps_sm.tile([P, 2 * NRC], f32, tag="sm", name="pfs")
            nc.tensor.transpose(pfs[:, :NRC], s8, ident[:NRC, :NRC])
            nc.tensor.transpose(pfs[:, NRC:], q8, ident[:NRC, :NRC])
            m8 = psm.tile([P, NRC], f32, tag="m8")
            nc.vector.tensor_scalar_mul(m8, pfs[:, :NRC], 1.0 / DV)
            v8 = psm.tile([P, NRC], f32, tag="v8")
            nc.vector.tensor_scalar_mul(v8, pfs[:, NRC:], 1.0 / DV)
            t8 = psm.tile([P, NRC], f32, tag="t8")
            nc.vector.tensor_tensor(t8, m8, m8, OP.mult)
            nc.vector.tensor_tensor(v8, v8, t8, OP.subtract)
            sd8 = psm.tile([P, NRC], f32, tag="sd8")
            nc.scalar.activation(sd8, v8, AF.Sqrt, bias=LN_EPS)
            rstd8 = psm.tile([P, NRC], f32, tag="rstd8")
            nc.vector.reciprocal(rstd8, sd8)
            mr8 = psm.tile([P, NRC], bf, tag="mr8")
            nc.vector.tensor_tensor(mr8, m8, rstd8, OP.mult)
            rstd8_b = psm.tile([P, NRC], bf, tag="rstd8b")
            nc.vector.tensor_copy(rstd8_b, rstd8)
            # replicate rstd and m*rstd to [128, 1024] bf (flip + K=1 matmul)
            reps_sb = []
            for idx, src in enumerate((rstd8_b, mr8)):
                pfl2 = ps_sm.tile([NRC, P], bf, tag="sm", name="pfl2")
                nc.tensor.transpose(pfl2, src, ident_b)
                sbt2 = psm.tile([NRC, P], bf, tag=f"sbt{idx}")
                nc.vector.tensor_copy(sbt2, pfl2)
                row2 = psm.tile([1, N], bf, tag=f"row{idx}")
                nc.sync.dma_start(
                    row2.rearrange("o (a b) -> o a b", a=NRC), sbt2)
                rps = ps_big.tile([P, N], f32, tag="big", name="rps")
                nc.tensor.matmul(rps[:, :DV], ones_one, row2[:, :DV],
                                 start=True, stop=True)
                nc.tensor.matmul(rps[:, DV:], ones_one, row2[:, DV:],
                                 start=True, stop=True)
                rep = prep.tile([P, N], bf, tag="irep", name=f"rep{idx}")
                nc.scalar.activation(rep[:, :DV], rps[:, :DV], AF.Copy)
                nc.vector.tensor_copy(rep[:, DV:], rps[:, DV:])
                reps_sb.append(rep)
            rstd_rep, mr_rep = reps_sb

            # normalize: O1T = (OT*rstd - m*rstd)*g0 + b0   (g0/b0 on parts)
            O1T = [po1.tile([P, N], bf, tag="o1t", name="o1t")
                   for _ in range(NCC)]
            for cc in range(NCC):
                t = pE.tile([P, N], bf, tag="E", name="ln0t")
                eng = nc.gpsimd if cc % 2 == 0 else nc.vector
                eng.tensor_tensor(t, OT[cc], rstd_rep, OP.mult)
                nc.vector.tensor_tensor(t, t, mr_rep, OP.subtract)
                nc.vector.tensor_scalar(O1T[cc], t, g0_col[:, cc:cc + 1],
                                        b0_col[:, cc:cc + 1], OP.mult, OP.add)

            # ---- FFN: O2T = O1T + relu(Wo@O1T + bo) ------------------------
            O2T = [po2.tile([P, N], bf, tag="o2t", name="o2t")
                   for _ in range(NCC)]
            for c2 in range(NCC):
                ps = ps_e.tile([P, N], f32, tag="eps", name="ffn_ps")
                for cc in range(NCC):
                    for hf in range(2):
                        nc.tensor.matmul(ps[:, hf * DV:(hf + 1) * DV],
                                         WoT[cc][:, c2 * P:(c2 + 1) * P],
                                         O1T[cc][:, hf * DV:(hf + 1) * DV],
                                         start=(cc == 0), stop=(cc == NCC - 1))
                t = pq.tile([P, N], bf, tag="ffn_t", name="ffn_t")
                nc.scalar.activation(t, ps, AF.Relu, bias=bo_col[:, c2:c2 + 1])
                nc.vector.tensor_tensor(O2T[c2], t, O1T[c2], OP.add)

            # ---- LN1 (row-major after xbar DMA-transpose) + store ----------
            for rc in range(NRC):
                psf = ps_sm.tile([P, DV], bf, tag="sm", name="ln1_psf")
                for cc in range(NCC):
                    nc.tensor.transpose(psf[:, cc * P:(cc + 1) * P],
                                        O2T[cc][:, rc * P:(rc + 1) * P],
                                        ident_b)
                mean = psm.tile([P, 1], f32, tag="ln1_mean")
                msc = pq.tile([P, DV], bf, tag="ffn_t", name="ln1_msc")
                nc.scalar.activation(msc, psf, AF.Copy, scale=1.0 / DV,
                                     accum_out=mean)
                xc = pout.tile([P, DV], bf, tag="ln1_xc")
                nc.vector.tensor_scalar_sub(xc, psf, mean)
                sqj = pq.tile([P, DV], bf, tag="ffn_t", name="ln1_sqj")
                ss = psm.tile([P, 1], f32, tag="ln1_ss")
                nc.scalar.activation(sqj, xc, AF.Square, accum_out=ss)
                nc.vector.tensor_scalar_mul(ss, ss, 1.0 / DV)
                sd = psm.tile([P, 1], f32, tag="ln1_sd")
                nc.scalar.activation(sd, ss, AF.Sqrt, bias=LN_EPS)
                rstd = psm.tile([P, 1], f32, tag="ln1_rstd")
                nc.vector.reciprocal(rstd, sd)
                ob = pout.tile([P, DV], f32, tag="ln1_out")
                nc.vector.scalar_tensor_tensor(ob, xc, rstd, g1_rep,
                                               OP.mult, OP.mult)
                nc.gpsimd.tensor_tensor(ob, ob, b1_rep, OP.add)
                nc.sync.dma_start(dout[rc * P:(rc + 1) * P, :], ob)

    nc.finalize()
    return nc


def kernel(**inputs):
    from concourse.bass_utils import run_bass_kernel_spmd

    if "nc" not in _CACHE:
        _CACHE["nc"] = _build()
    nc = _CACHE["nc"]

    Q = np.ascontiguousarray(np.asarray(inputs["Q"], dtype=np.float32))
    shared = {k: np.ascontiguousarray(np.asarray(inputs[k], dtype=np.float32))
              for k in ("Wq", "bq", "Wo", "bo", "g0", "b0", "g1", "b1")}
    in_maps = [dict(Q=np.ascontiguousarray(Q[b]), **shared) for b in range(B)]

    res = run_bass_kernel_spmd(nc, in_maps, core_ids=list(range(B)),
                               **_CACHE.get("run_kwargs", {}))
    _CACHE["last_result"] = res
    return np.stack([r["out"] for r in res.results], axis=0)
